# revision 9
# baseline (speedup 1.0000x reference)
"""Embedding lookup (gather rows of W.T by index, + bias) on 8 TRN2 cores.

Strategy: vocab-sharded ("row-parallel") embedding. The bias is folded into
the table on the host (out = (W.T + b)[x], exactly). Each core owns a
12500-row shard of the 100000-row table; the host routes each token index to
its owning core (a sort by index value, which both groups tokens by shard and
orders gather reads for HBM row locality), the device does the entire data
movement: an indexed-gather DMA of 256 B rows from HBM -> SBUF, then streams
the gathered rows back out to HBM. The host applies the inverse routing
permutation to assemble the full [4096, 200, 64] output.

Device kernel (SPMD on 8 cores, identical program):
  - idx tile [128, N_PAD/16] int16 loaded to SBUF once (dma_gather expects
    index i of a chunk at partition i%16, free slot i//16, replicated on all
    8 16-partition groups).
  - 102 chunks of 1024 indices (single_packet dma_gather caps at 64
    descriptors/lane = 1024 idxs). Chunks rotate over 4 SWDGE queues (4
    descriptor rings -> up to 4 gathers' descriptors in flight) and 8 SBUF
    buffers [128, 8, 64] f32; nc.sync (HWDGE) streams each gathered buffer
    to the output HBM tensor, overlapping subsequent gathers.
"""

import numpy as np

import concourse.bass as bass
import concourse.bacc as bacc
import concourse.mybir as mybir
from concourse.library_config import mlp
from concourse.bass_utils import run_bass_kernel_spmd

VOCAB = 100000
E = 64                  # embedding dim; 64 * 4 B = 256 B rows
N_CORES = 8
SHARD = VOCAB // N_CORES            # 12500 rows per core (< int16 max)
C = 1024                # indices per dma_gather (single_packet limit)
NCH = 102               # chunks per core
N_PAD = C * NCH         # 104448 padded indices per core (mean 102400)
F = C // 128            # free slots per chunk in the SBUF gather buffer
NB = 16                 # rotating gather buffers
NQ = 4                  # SWDGE queues (one descriptor ring each)
CS = C // 16            # idx-tile columns per chunk
IDX_PIECES = [0, 26, 52, 77, NCH]  # idx tile loads in 4 pieces (chunk bounds)

_compiled = None


def _build():
    nc = bacc.Bacc("TRN2", num_swdge_queues=NQ)
    w_hbm = nc.dram_tensor("w", [SHARD, E], mybir.dt.float32, kind="ExternalInput")
    idx_hbm = nc.dram_tensor(
        "idx", [128, N_PAD // 16], mybir.dt.int16, kind="ExternalInput"
    )
    out_hbm = nc.dram_tensor(
        "out", [NCH, 128, F * E], mybir.dt.float32, kind="ExternalOutput"
    )

    import contextlib

    with contextlib.ExitStack() as stack:
        block = stack.enter_context(nc.Block())
        idxs_sbuf = stack.enter_context(
            nc.sbuf_tensor("idxs_sbuf", [128, N_PAD // 16], mybir.dt.int16)
        )
        bufs = [
            stack.enter_context(
                nc.sbuf_tensor(f"buf{j}", [128, F, E], mybir.dt.float32)
            )
            for j in range(NB)
        ]
        isems = [stack.enter_context(nc.semaphore(f"isem{p}")) for p in range(4)]
        gsems = [stack.enter_context(nc.semaphore(f"g{j}")) for j in range(NB)]
        wsems = [stack.enter_context(nc.semaphore(f"ws{j}")) for j in range(NB)]

        @block.gpsimd
        def _(g: bass.BassGpSimd):
            # idx pieces via SWDGE (deterministic +16/DMA); drains overlap
            # the library load that follows
            for p in range(4):
                a, b = IDX_PIECES[p] * CS, IDX_PIECES[p + 1] * CS
                g.dma_start(idxs_sbuf[:, a:b], idx_hbm[:, a:b]).then_inc(
                    isems[p], 16
                )
            g.load_library(mlp)
            for k in range(NCH):
                j = k % NB
                if k in IDX_PIECES:
                    p = IDX_PIECES.index(k)
                    g.wait_ge(isems[p], 16)
                if k >= NB:
                    # WAR: wait for write-out of the chunk that last used slot j
                    g.wait_ge(wsems[j], 16 * ((k - NB) // NB + 1))
                g.dma_gather(
                    bufs[j][:],
                    w_hbm[:],
                    idxs_sbuf[:, k * CS : (k + 1) * CS],
                    C,
                    C,
                    E,
                    queue_num=k % NQ,
                ).then_inc(gsems[j], 16)

        # write-outs split across the two HWDGE engines (SP=even, ACT=odd
        # chunks) to double issue streams and reduce head-of-line blocking
        def _writer(eng, parity):
            for k in range(parity, NCH, 2):
                j = k % NB
                eng.wait_ge(gsems[j], 16 * (k // NB + 1))
                eng.dma_start(out_hbm[k], bufs[j][:]).then_inc(wsems[j], 16)
            for j in range(parity, NB, 2):
                ks = [k for k in range(NCH) if k % NB == j]
                eng.wait_ge(wsems[j], 16 * len(ks))

        @block.sync
        def _(s: bass.BassEngine):
            _writer(s, 0)

        @block.scalar
        def _(sc: bass.BassEngine):
            _writer(sc, 1)

    nc.compile()
    return nc


def _get_compiled():
    global _compiled
    if _compiled is None:
        _compiled = _build()
    return _compiled


def _run(x, W, b, trace=False):
    x = np.asarray(x)
    W = np.asarray(W, dtype=np.float32)
    b = np.asarray(b, dtype=np.float32)
    orig_shape = x.shape
    xf = np.ascontiguousarray(x).reshape(-1).astype(np.int64)
    n_tok = xf.shape[0]

    table = W.T + b  # [VOCAB, E]; bias folded in exactly (fp32 add, matches ref)

    # route tokens: sort by index value == group by owning core + ascending
    # rows within each core (sequential-ish HBM reads)
    order = np.argsort(xf, kind="stable")
    sx = xf[order]
    counts = np.bincount(sx // SHARD, minlength=N_CORES)
    starts = np.concatenate(([0], np.cumsum(counts)))[:N_CORES]

    in_maps = []
    overflow = []  # (core, positions) handled on host if a bucket > N_PAD
    for c in range(N_CORES):
        n_c = int(counts[c])
        pos_c = order[starts[c] : starts[c] + n_c]
        if n_c > N_PAD:
            overflow.append(pos_c[N_PAD:])
            pos_c = pos_c[:N_PAD]
            n_c = N_PAD
        loc = (xf[pos_c] - c * SHARD).astype(np.int16)
        pad = np.zeros(N_PAD, dtype=np.int16)
        pad[:n_c] = loc
        # chunk k, index i -> partition i%16, column k*CS + i//16; replicate x8
        tile16 = pad.reshape(NCH, CS, 16).transpose(2, 0, 1).reshape(16, -1)
        idx_tile = np.tile(tile16, (8, 1))
        w_shard = np.ascontiguousarray(table[c * SHARD : (c + 1) * SHARD])
        in_maps.append({"w": w_shard, "idx": idx_tile})

    nc = _get_compiled()
    br = run_bass_kernel_spmd(
        nc, in_maps, core_ids=list(range(N_CORES)), trace=trace
    )

    out_full = np.empty((n_tok, E), dtype=np.float32)
    for c in range(N_CORES):
        n_c = min(int(counts[c]), N_PAD)
        pos_c = order[starts[c] : starts[c] + n_c]
        dev = br.results[c]["out"].reshape(NCH, 128, F, E)
        # gathered row i of chunk k lives at [k, i%128, i//128]
        rows = dev.transpose(0, 2, 1, 3).reshape(N_PAD, E)
        out_full[pos_c] = rows[:n_c]
    for pos in overflow:  # statistically never taken; exact host fallback
        out_full[pos] = table[xf[pos]]

    return out_full.reshape(*orig_shape, E), br


def kernel(x, W, b):
    out, _ = _run(x, W, b, trace=False)
    return out


# revision 10
# speedup vs baseline: 1.0043x; 1.0043x over previous
"""Embedding lookup (gather rows of W.T by index, + bias) on 8 TRN2 cores.

Strategy: vocab-sharded ("row-parallel") embedding. The bias is folded into
the table on the host (out = (W.T + b)[x], exactly). Each core owns a
12500-row shard of the 100000-row table; the host routes each token index to
its owning core (a sort by index value, which both groups tokens by shard and
orders gather reads for HBM row locality), the device does the entire data
movement: an indexed-gather DMA of 256 B rows from HBM -> SBUF, then streams
the gathered rows back out to HBM. The host applies the inverse routing
permutation to assemble the full [4096, 200, 64] output.

Device kernel (SPMD on 8 cores, identical program):
  - idx tile [128, N_PAD/16] int16 loaded to SBUF once (dma_gather expects
    index i of a chunk at partition i%16, free slot i//16, replicated on all
    8 16-partition groups).
  - 102 chunks of 1024 indices (single_packet dma_gather caps at 64
    descriptors/lane = 1024 idxs). Chunks rotate over 4 SWDGE queues (4
    descriptor rings -> up to 4 gathers' descriptors in flight) and 8 SBUF
    buffers [128, 8, 64] f32; nc.sync (HWDGE) streams each gathered buffer
    to the output HBM tensor, overlapping subsequent gathers.
"""

import numpy as np

import concourse.bass as bass
import concourse.bacc as bacc
import concourse.mybir as mybir
from concourse.library_config import mlp
from concourse.bass_utils import run_bass_kernel_spmd

VOCAB = 100000
E = 64                  # embedding dim; 64 * 4 B = 256 B rows
N_CORES = 8
SHARD = VOCAB // N_CORES            # 12500 rows per core (< int16 max)
C = 1024                # indices per dma_gather (single_packet limit)
NCH = 102               # chunks per core
N_PAD = C * NCH         # 104448 padded indices per core (mean 102400)
F = C // 128            # free slots per chunk in the SBUF gather buffer
NB = 16                 # rotating gather buffers
NQ = 4                  # SWDGE queues (one descriptor ring each)
CS = C // 16            # idx-tile columns per chunk
IDX_PIECES = [0, 26, 52, 77, NCH]  # idx tile loads in 4 pieces (chunk bounds)

_compiled = None


def _build():
    nc = bacc.Bacc("TRN2", num_swdge_queues=NQ, dynamic_dma_scratch_size=32768)
    w_hbm = nc.dram_tensor("w", [SHARD, E], mybir.dt.float32, kind="ExternalInput")
    idx_hbm = nc.dram_tensor(
        "idx", [128, N_PAD // 16], mybir.dt.int16, kind="ExternalInput"
    )
    out_hbm = nc.dram_tensor(
        "out", [NCH, 128, F * E], mybir.dt.float32, kind="ExternalOutput"
    )

    import contextlib

    with contextlib.ExitStack() as stack:
        block = stack.enter_context(nc.Block())
        idxs_sbuf = stack.enter_context(
            nc.sbuf_tensor("idxs_sbuf", [128, N_PAD // 16], mybir.dt.int16)
        )
        bufs = [
            stack.enter_context(
                nc.sbuf_tensor(f"buf{j}", [128, F, E], mybir.dt.float32)
            )
            for j in range(NB)
        ]
        isems = [stack.enter_context(nc.semaphore(f"isem{p}")) for p in range(4)]
        gsems = [stack.enter_context(nc.semaphore(f"g{j}")) for j in range(NB)]
        wsems = [stack.enter_context(nc.semaphore(f"ws{j}")) for j in range(NB)]

        @block.gpsimd
        def _(g: bass.BassGpSimd):
            # idx pieces via SWDGE (deterministic +16/DMA); drains overlap
            # the library load that follows
            for p in range(4):
                a, b = IDX_PIECES[p] * CS, IDX_PIECES[p + 1] * CS
                g.dma_start(idxs_sbuf[:, a:b], idx_hbm[:, a:b]).then_inc(
                    isems[p], 16
                )
            g.load_library(mlp)
            for k in range(NCH):
                j = k % NB
                if k in IDX_PIECES:
                    p = IDX_PIECES.index(k)
                    g.wait_ge(isems[p], 16)
                if k >= NB:
                    # WAR: wait for write-out of the chunk that last used slot j
                    g.wait_ge(wsems[j], 16 * ((k - NB) // NB + 1))
                g.dma_gather(
                    bufs[j][:],
                    w_hbm[:],
                    idxs_sbuf[:, k * CS : (k + 1) * CS],
                    C,
                    C,
                    E,
                    queue_num=k % NQ,
                ).then_inc(gsems[j], 16)

        # write-outs split across the two HWDGE engines (SP=even, ACT=odd
        # chunks) to double issue streams and reduce head-of-line blocking
        def _writer(eng, parity):
            for k in range(parity, NCH, 2):
                j = k % NB
                eng.wait_ge(gsems[j], 16 * (k // NB + 1))
                eng.dma_start(out_hbm[k], bufs[j][:]).then_inc(wsems[j], 16)
            for j in range(parity, NB, 2):
                ks = [k for k in range(NCH) if k % NB == j]
                eng.wait_ge(wsems[j], 16 * len(ks))

        @block.sync
        def _(s: bass.BassEngine):
            _writer(s, 0)

        @block.scalar
        def _(sc: bass.BassEngine):
            _writer(sc, 1)

    nc.compile()
    return nc


def _get_compiled():
    global _compiled
    if _compiled is None:
        _compiled = _build()
    return _compiled


def _run(x, W, b, trace=False):
    x = np.asarray(x)
    W = np.asarray(W, dtype=np.float32)
    b = np.asarray(b, dtype=np.float32)
    orig_shape = x.shape
    xf = np.ascontiguousarray(x).reshape(-1).astype(np.int64)
    n_tok = xf.shape[0]

    table = W.T + b  # [VOCAB, E]; bias folded in exactly (fp32 add, matches ref)

    # route tokens: sort by index value == group by owning core + ascending
    # rows within each core (sequential-ish HBM reads)
    order = np.argsort(xf, kind="stable")
    sx = xf[order]
    counts = np.bincount(sx // SHARD, minlength=N_CORES)
    starts = np.concatenate(([0], np.cumsum(counts)))[:N_CORES]

    in_maps = []
    overflow = []  # (core, positions) handled on host if a bucket > N_PAD
    for c in range(N_CORES):
        n_c = int(counts[c])
        pos_c = order[starts[c] : starts[c] + n_c]
        if n_c > N_PAD:
            overflow.append(pos_c[N_PAD:])
            pos_c = pos_c[:N_PAD]
            n_c = N_PAD
        loc = (xf[pos_c] - c * SHARD).astype(np.int16)
        pad = np.zeros(N_PAD, dtype=np.int16)
        pad[:n_c] = loc
        # chunk k, index i -> partition i%16, column k*CS + i//16; replicate x8
        tile16 = pad.reshape(NCH, CS, 16).transpose(2, 0, 1).reshape(16, -1)
        idx_tile = np.tile(tile16, (8, 1))
        w_shard = np.ascontiguousarray(table[c * SHARD : (c + 1) * SHARD])
        in_maps.append({"w": w_shard, "idx": idx_tile})

    nc = _get_compiled()
    br = run_bass_kernel_spmd(
        nc, in_maps, core_ids=list(range(N_CORES)), trace=trace
    )

    out_full = np.empty((n_tok, E), dtype=np.float32)
    for c in range(N_CORES):
        n_c = min(int(counts[c]), N_PAD)
        pos_c = order[starts[c] : starts[c] + n_c]
        dev = br.results[c]["out"].reshape(NCH, 128, F, E)
        # gathered row i of chunk k lives at [k, i%128, i//128]
        rows = dev.transpose(0, 2, 1, 3).reshape(N_PAD, E)
        out_full[pos_c] = rows[:n_c]
    for pos in overflow:  # statistically never taken; exact host fallback
        out_full[pos] = table[xf[pos]]

    return out_full.reshape(*orig_shape, E), br


def kernel(x, W, b):
    out, _ = _run(x, W, b, trace=False)
    return out
